# revision 7
# baseline (speedup 1.0000x reference)
"""Trainium2 Bass kernel for Gaussian KDE on a separable 2D grid.

out[b,i,j] = (1/Z_b) * sum_n exp(-||s_bn - (g_i, g_j)||^2 / (2h^2))

The evaluation grid is a meshgrid, so the Gaussian factorizes:
    exp(-((sx-g_i)^2 + (sy-g_j)^2)*inv) = fx[n,i] * fy[n,j]
    out_unnorm[b] = fx[b]^T @ fy[b]   (contraction over N=4096 on TensorE)

Per core (8 cores, batch b = core % 4, pairs redundant):
  1. fused K=6 aug matmul per 128-sample chunk: with lhsT rows
     [1,sx,sx^2,1,sy,sy^2] and block-diagonal rhs
     [[gx^2;-2gx;1;0;0;0] | [0;0;0;gy^2;-2gy;1]] one matmul yields
     [dx^2 | dy^2] (128n x 256) in PSUM.
  2. ACT exp (scale=-inv) over (128,1024) PSUM (4 chunks) -> SBUF f
     (128p = n%128, 8192f; chunk c at cols [256c, 256c+256) = [fx_c|fy_c])
  3. big matmul accumulating over 32 K-chunks -> PSUM grid (128i, 128j)
  4. norm: free reduce + ones-matmul partition reduce + reciprocal +
     ones-matmul broadcast + per-partition scale; DMA out.
"""

import numpy as np

B, N, H, W = 4, 4096, 128, 128
BANDWIDTH = 0.1
INV = 1.0 / (2.0 * BANDWIDTH * BANDWIDTH)  # 50.0
NCHUNK = N // 128  # 32
NTILE = NCHUNK // 4  # 8 aug psum tiles, 4 chunks each
N_CORES = 8

_cache = {}


def _split_excess_waits(nc, max_waits=1):
    """walrus on this image rejects >1 sem wait per instruction
    ('Too many sync wait commands'); hoist excess waits onto NOPs."""
    import concourse.mybir as mybir

    ctr = 0
    for f in nc.m.functions:
        for blk in f.blocks:
            out = []
            changed = False
            for inst in blk.instructions:
                si = inst.sync_info
                if si is not None and len(si.on_wait) > max_waits:
                    waits = list(si.on_wait)
                    excess = waits[max_waits:]
                    for k in range(0, len(excess), max_waits):
                        ctr += 1
                        out.append(
                            mybir.InstNoOp(
                                name=f"{inst.name}-ws{ctr}",
                                sync_info=mybir.SyncInfo(
                                    on_wait=excess[k : k + max_waits], on_update=[]
                                ),
                                bass_nofuse=True,
                                engine=inst.engine,
                            )
                        )
                    inst.sync_info = mybir.SyncInfo(
                        on_wait=waits[:max_waits], on_update=list(si.on_update)
                    )
                    changed = True
                out.append(inst)
            if changed:
                blk.instructions = out


K_AUG = 14  # 7 bf16 hi/lo-split rows per axis (see _prep_in_maps)


def _build():
    import concourse.bass as bass
    import concourse.mybir as mybir
    import concourse.tile as tile

    f32 = mybir.dt.float32
    bf16 = mybir.dt.bfloat16
    nc = bass.Bass("TRN2", target_bir_lowering=False, debug=False, num_devices=N_CORES)

    SXY = nc.dram_tensor("sxy", [K_AUG, N], bf16, kind="ExternalInput")
    G = nc.dram_tensor("g", [K_AUG, 2 * H], bf16, kind="ExternalInput")
    OUT = nc.dram_tensor("out", [H, W], f32, kind="ExternalOutput")

    Exp = mybir.ActivationFunctionType.Exp

    with tile.TileContext(nc) as tc:
        with (
            tc.tile_pool(name="cst", bufs=1) as cst,
            tc.tile_pool(name="sb", bufs=1) as sb,
            tc.tile_pool(name="ps", bufs=3, space="PSUM") as ps,
            tc.tile_pool(name="pso", bufs=1, space="PSUM") as pso,
            tc.tile_pool(name="psn", bufs=1, space="PSUM") as psn,
        ):
            sxy_sb = cst.tile([K_AUG, N], bf16, tag="sxy")
            g_sb = cst.tile([K_AUG, 2 * H], bf16, tag="g")
            for q in range(4):
                sl = slice(q * (N // 4), (q + 1) * (N // 4))
                nc.sync.dma_start(sxy_sb[:, sl], SXY.ap()[:, sl])
            nc.sync.dma_start(g_sb[:], G.ap()[:])

            # f: chunk c occupies cols [256c, 256c+128) = fx_c, [256c+128, 256c+256) = fy_c
            f_sb = sb.tile([128, 2 * N], bf16, tag="f")

            for t in range(NTILE):
                pf = ps.tile([128, 1024], f32, tag="aug")
                for q in range(4):
                    c = t * 4 + q
                    nc.tensor.matmul(
                        pf[:, q * 256 : (q + 1) * 256],
                        sxy_sb[:, c * 128 : (c + 1) * 128],
                        g_sb[:],
                        start=True,
                        stop=True,
                    )
                nc.scalar.activation(
                    f_sb[:, t * 1024 : (t + 1) * 1024],
                    pf[:],
                    Exp,
                    bias=0.0,
                    scale=-INV,
                )

            # big matmul: out[i,j] = sum_n fx[n,i] fy[n,j], accumulated in PSUM
            po = pso.tile([H, W], f32, tag="out")
            for c in range(NCHUNK):
                nc.tensor.matmul(
                    po[:],
                    f_sb[:, c * 256 : c * 256 + 128],
                    f_sb[:, c * 256 + 128 : c * 256 + 256],
                    start=(c == 0),
                    stop=(c == NCHUNK - 1),
                )

            # normalization: Z = sum_ij po; out = po / Z
            ones_col = cst.tile([128, 1], f32, tag="ones_col")
            ones_row = cst.tile([1, 128], f32, tag="ones_row")
            nc.vector.memset(ones_col[:], 1.0)
            nc.vector.memset(ones_row[:], 1.0)

            colsum = sb.tile([H, 1], f32, tag="colsum")
            nc.vector.tensor_reduce(
                colsum[:], po[:], axis=mybir.AxisListType.X, op=mybir.AluOpType.add
            )
            pn = psn.tile([1, 1], f32, tag="nrm")
            nc.tensor.matmul(pn[:], colsum[:], ones_col[:], start=True, stop=True)
            rn = sb.tile([1, 1], f32, tag="rnorm")
            nc.vector.reciprocal(rn[:], pn[:])
            pb = psn.tile([128, 1], f32, tag="nrm")
            nc.tensor.matmul(pb[:], ones_row[:], rn[:], start=True, stop=True)
            rn_bc = sb.tile([128, 1], f32, tag="rnbc")
            nc.vector.tensor_copy(rn_bc[:], pb[:])

            out_sb = sb.tile([H, W], f32, tag="outsb")
            nc.vector.tensor_scalar_mul(out_sb[:], po[:], rn_bc[:])
            nc.sync.dma_start(OUT.ap()[:], out_sb[:])

    _split_excess_waits(nc)
    return nc


def _split_bf16(x):
    """x (fp32) -> (hi, lo) bf16 with hi + lo ~= x to ~16 mantissa bits."""
    import ml_dtypes

    hi = x.astype(ml_dtypes.bfloat16)
    lo = (x - hi.astype(np.float32)).astype(ml_dtypes.bfloat16)
    return hi, lo


def _prep_in_maps(samples, locations):
    """Build the K=14 split-precision augmented operands.

    Per axis (7 rows k: lhs row a_k[n], rhs row b_k[m]):
      d2 = s^2 - 2 s g + g^2 with every factor bf16 hi/lo split:
        (1, g2h) (1, g2l)          g^2
        (s2h, 1) (s2l, 1)          s^2
        (sh, -2gh) (sh, -2gl) (sl, -2gh)   -2 s g  (sl*gl dropped, ~2^-18)
    bf16*bf16 is exact in fp32 PSUM accumulation, so d2 error ~1e-4 abs.
    """
    import ml_dtypes

    bf16 = ml_dtypes.bfloat16
    samples = np.asarray(samples, np.float32)
    locations = np.asarray(locations, np.float32)
    gi = np.ascontiguousarray(locations[:, 0, 0])  # grid values along i (x)
    gj = np.ascontiguousarray(locations[0, :, 1])  # grid values along j (y)

    g = np.zeros((K_AUG, 2 * H), bf16)
    for ax, gv in ((0, gi), (1, gj)):
        g2h, g2l = _split_bf16(gv * gv)
        gh, gl = _split_bf16(-2.0 * gv)
        r = 7 * ax
        cols = slice(0, H) if ax == 0 else slice(H, 2 * H)
        g[r + 0, cols] = g2h
        g[r + 1, cols] = g2l
        g[r + 2, cols] = 1.0
        g[r + 3, cols] = 1.0
        g[r + 4, cols] = gh
        g[r + 5, cols] = gl
        g[r + 6, cols] = gh

    in_maps = []
    for c in range(N_CORES):
        b = c % B
        sxy = np.zeros((K_AUG, N), bf16)
        for ax in range(2):
            s = samples[b, :, ax]
            s2h, s2l = _split_bf16(s * s)
            sh, sl = _split_bf16(s)
            r = 7 * ax
            sxy[r + 0] = 1.0
            sxy[r + 1] = 1.0
            sxy[r + 2] = s2h
            sxy[r + 3] = s2l
            sxy[r + 4] = sh
            sxy[r + 5] = sh
            sxy[r + 6] = sl
        in_maps.append({"sxy": sxy, "g": g})
    return in_maps


def kernel(samples: np.ndarray, locations: np.ndarray) -> np.ndarray:
    from concourse.bass_utils import run_bass_kernel_spmd

    if "nc" not in _cache:
        _cache["nc"] = _build()
    nc = _cache["nc"]

    in_maps = _prep_in_maps(samples, locations)
    res = run_bass_kernel_spmd(nc, in_maps, core_ids=list(range(N_CORES)))
    out = np.stack([res.results[b]["out"] for b in range(B)]).astype(np.float32)
    return out
